# revision 23
# baseline (speedup 1.0000x reference)
"""Trainium2 Bass kernel for Ac4kAttentionOp (int8 q/k + fp8e4m3 v quantized attention).

Shapes: q,k,v [B=2, H=16, N=2048, D=64] fp32 -> out [2,16,2048,64] fp32.
Sharding: 32 (B,H) heads split 4-per-core across 8 NeuronCores; no collectives.

Math (mirrors the reference exactly up to fp32 rounding order):
  k <- k - mean_N(k)
  qq = round(q / sf_q), sf_q = max(amax_D(q)/127, eps)      (per token)
  kq = round(k / sf_k), sf_k = max(amax_D(k)/127, eps)      (per token)
  vq = fp8e4m3(v / sf_v), sf_v = max(amax_N(v)/(448/2.25), eps)  (per channel)
  s^T[m,nq] = sum_d kq[m,d] * (qq[nq,d]*sf_q[nq]*sm) ;  p^T = exp(sf_k[m] * s^T)
  outT[d,nq] = sum_m vq[m,d] * p^T[m,nq] ; denom = ones-column of vq_aug
  out[nq,d] = outT[d,nq] * sf_v[d] / denom[nq]

Engine mapping: integer-valued operands ride bf16 (exact); fp32-valued moving
operands ride float32r (1 cyc/row at free>=256). Per-query dequant scale is
pre-folded into the QK moving operand; per-key scale applied via the ACT Exp
per-partition scale; per-channel v scale applied per-partition on outT.
"""
import math
from contextlib import ExitStack

import numpy as np

import concourse.bass as bass
import concourse.tile as tile
from concourse import mybir
from concourse.masks import make_identity

B, H, N, D = 2, 16, 2048, 64
NCORES = 8
HEADS_PER_CORE = (B * H) // NCORES          # 4
SM_SCALE = 1.0 / math.sqrt(D)               # 0.125 (exact power of 2)
MAGIC = 12582912.0                          # 1.5*2^23: fp32 RNE integer round
INT8_MAX = 127.0
F8_AMAX_DIV = float(np.float32(448.0) / np.float32(2.25))  # FP8_MAX / MAX_SCALE
EPS = 1e-8

f32 = mybir.dt.float32
f32r = mybir.dt.float32r
bf16 = mybir.dt.bfloat16
f16 = mybir.dt.float16
f8e4 = mybir.dt.float8e4
ALU = mybir.AluOpType
ACTF = mybir.ActivationFunctionType


def _bc(t: bass.AP, dims, off: int = 0) -> bass.AP:
    """Build a broadcast/restrided view of a tile AP (off in elements)."""
    return bass.AP(tensor=t.tensor, offset=t.offset + off, ap=dims)


def build_attention(nc: bass.Bass, heads: int = HEADS_PER_CORE, n: int = N,
                    stage: str = "full", bench_loops: int = 0):
    T = n // 128          # token tiles per head
    C = T // 2            # 128-wide transpose chunks
    NQH = n // 2          # query-half width (PSUM budget)
    q_d = nc.dram_tensor("q", [heads, n, D], f32, kind="ExternalInput").ap()
    k_d = nc.dram_tensor("k", [heads, n, D], f32, kind="ExternalInput").ap()
    v_d = nc.dram_tensor("v", [heads, n, D], f32, kind="ExternalInput").ap()
    o_d = nc.dram_tensor("out", [heads, n, D], f32, kind="ExternalOutput").ap()

    with tile.TileContext(nc) as tc, ExitStack() as ctx:
        singles = ctx.enter_context(tc.tile_pool(name="singles", bufs=1))
        loads = ctx.enter_context(tc.tile_pool(name="loads", bufs=2))
        work = ctx.enter_context(tc.tile_pool(name="work", bufs=2))
        scales = ctx.enter_context(tc.tile_pool(name="scales", bufs=2))
        small = ctx.enter_context(tc.tile_pool(name="small", bufs=4))
        opnds = ctx.enter_context(tc.tile_pool(name="opnds", bufs=2))
        pbuf = ctx.enter_context(tc.tile_pool(name="pbuf", bufs=3))
        obuf = ctx.enter_context(tc.tile_pool(name="obuf", bufs=2))
        ostore = ctx.enter_context(tc.tile_pool(name="ostore", bufs=4))
        ps_s = ctx.enter_context(tc.tile_pool(name="ps_s", bufs=2, space="PSUM"))
        ps_o = ctx.enter_context(tc.tile_pool(name="ps_o", bufs=1, space="PSUM"))
        ps_t = ctx.enter_context(tc.tile_pool(name="ps_t", bufs=2, space="PSUM"))

        ident_f = singles.tile([128, 128], f32)
        make_identity(nc, ident_f)
        ident_h = singles.tile([128, 128], f16)
        make_identity(nc, ident_h)
        ones_row = singles.tile([1, 128], f32)
        nc.gpsimd.memset(ones_row, 1.0)
        # constant [128,128] of 1/n: k-mean matmul weights (exact 2^-11 scale)
        invn_sq = singles.tile([128, 128], f32)
        nc.gpsimd.memset(invn_sq, 1.0 / n)

        if bench_loops:
            ctx.enter_context(tc.For_i(0, bench_loops, 1))

        for h in range(heads):
            # ---------------- loads: [n, D] -> [128, T, D] ----------------
            q_sb = loads.tile([128, T, D], f32, tag="q_sb")
            nc.sync.dma_start(out=q_sb, in_=q_d[h].rearrange("(t p) d -> p t d", p=128))
            k_sb = loads.tile([128, T, D], f32, tag="k_sb")
            nc.sync.dma_start(out=k_sb, in_=k_d[h].rearrange("(t p) d -> p t d", p=128))
            v_sb = loads.tile([128, T, D], f32, tag="v_sb")
            nc.sync.dma_start(out=v_sb, in_=v_d[h].rearrange("(t p) d -> p t d", p=128))

            # ---------------- v prep: per-channel fp8 quant ----------------
            # partial amax over t within each partition: [128, D]
            amax_vp = work.tile([128, D], f32, tag="amax_vp")
            nc.vector.tensor_reduce(
                out=amax_vp,
                in_=_bc(v_sb, [v_sb.ap[0], [1, D], [D, T]]),
                axis=mybir.AxisListType.X, op=ALU.max, apply_absolute_value=True)
            # fold across partitions via PE transpose, then reduce free
            vt_ps = ps_t.tile([D, 128], f32, tag="pst")
            nc.tensor.transpose(vt_ps, amax_vp, ident_f)
            amax_vT = scales.tile([D, 1], f32, tag="amax_vT")
            nc.vector.tensor_reduce(out=amax_vT, in_=vt_ps,
                                    axis=mybir.AxisListType.X, op=ALU.max)
            sf_vT = scales.tile([D, 1], f32, tag="sf_vT")
            nc.vector.tensor_scalar(out=sf_vT, in0=amax_vT,
                                    scalar1=1.0 / F8_AMAX_DIV, scalar2=EPS,
                                    op0=ALU.mult, op1=ALU.max)
            rsf_vT = scales.tile([D, 1], f32, tag="rsf_vT")
            nc.vector.reciprocal(rsf_vT, sf_vT)
            # sfv65: [65,1] column for the outT scale (row 64 = 1.0 for denom)
            sfv65 = scales.tile([65, 1], f32, tag="sfv65")
            nc.gpsimd.memset(sfv65, 1.0)
            nc.vector.tensor_copy(sfv65[0:D, :], sf_vT)
            # rsf broadcast to [128, D] via rank-1 matmul (ones_row.T @ rsf_row)
            rsf_row = small.tile([1, D], f32, tag="rsf_row")
            nc.sync.dma_start(out=rsf_row, in_=rsf_vT)
            rsf_bps = ps_t.tile([128, D], f32, tag="pst")
            nc.tensor.matmul(rsf_bps, ones_row, rsf_row, start=True, stop=True)
            rsf_b = small.tile([128, D], f32, tag="rsf_b")
            nc.vector.tensor_copy(rsf_b, rsf_bps)
            # vq = fp8(v * rsf) ; vq_aug [128, T, D+1] bf16 with ones column
            vq_pre = work.tile([128, T, D], f32, tag="vq_pre")
            nc.vector.tensor_mul(vq_pre, v_sb,
                                 _bc(rsf_b, [rsf_b.ap[0], [0, T], [1, D]]))
            vq_f8 = work.tile([128, T, D], f8e4, tag="vq_f8")
            nc.vector.tensor_copy(vq_f8, vq_pre)
            vq_aug = opnds.tile([128, T, D + 1], f16, tag="vq_aug")
            nc.vector.tensor_copy(vq_aug[:, :, 0:D], vq_f8)
            nc.gpsimd.memset(vq_aug[:, :, D:D + 1], 1.0)

            # ---------------- k prep: mean-sub + int8 quant + transpose ----
            # meanb_ps[p, d] = sum_t sum_tok (1/n) * k[tok, d]: the 1/n weight
            # matrix makes every output row the (broadcast) per-d mean.
            meanb_ps = ps_t.tile([128, D], f32, tag="pst")
            for t in range(T):
                nc.tensor.matmul(meanb_ps, invn_sq, k_sb[:, t, :],
                                 start=(t == 0), stop=(t == T - 1))
            meanb = small.tile([128, D], f32, tag="meanb")
            nc.vector.tensor_copy(meanb, meanb_ps)

            ks = work.tile([128, T, D], f32, tag="ks")
            nc.vector.tensor_sub(ks, k_sb,
                                 _bc(meanb, [meanb.ap[0], [0, T], [1, D]]))

            def quant_int8(x_sb, tagpfx):
                """per-token int8 quantize: returns (q_rounded_f32, sf [128,T])."""
                amax = scales.tile([128, T], f32, tag=tagpfx + "amax")
                nc.vector.tensor_reduce(out=amax, in_=x_sb,
                                        axis=mybir.AxisListType.X, op=ALU.max,
                                        apply_absolute_value=True)
                sf = scales.tile([128, T], f32, tag=tagpfx + "sf")
                nc.vector.tensor_scalar(out=sf, in0=amax,
                                        scalar1=1.0 / INT8_MAX, scalar2=EPS,
                                        op0=ALU.mult, op1=ALU.max)
                rsf = scales.tile([128, T], f32, tag=tagpfx + "rsf")
                nc.vector.reciprocal(rsf, sf)
                xq = work.tile([128, T, D], f32, tag=tagpfx + "xq")
                nc.vector.tensor_mul(xq, x_sb,
                                     _bc(rsf, [rsf.ap[0], [1, T], [0, D]]))
                # RNE integer round: (x + MAGIC) - MAGIC
                nc.vector.tensor_scalar(out=xq, in0=xq,
                                        scalar1=MAGIC, scalar2=MAGIC,
                                        op0=ALU.add, op1=ALU.subtract)
                return xq, sf

            kq, sf_k = quant_int8(ks, "k")
            kq_bf = work.tile([128, T, D], f16, tag="kq_bf")
            nc.vector.tensor_copy(kq_bf, kq)
            kqT_st = work.tile([128, C, 128], f16, tag="kqT_st")
            kqT = opnds.tile([64, T, 128], f16, tag="kqT")
            for c in range(C):
                tp = ps_t.tile([128, 128], f16, tag="pst")
                nc.tensor.transpose(tp, kq_bf[:, 2 * c:2 * c + 2, :], ident_h)
                nc.vector.tensor_copy(kqT_st[:, c, :], tp)
            # parity split in 2 strided sb->sb DMAs (keeps consumer waits small)
            nc.sync.dma_start(out=_bc(kqT, [kqT.ap[0], [2 * 128, C], [1, 128]]),
                              in_=kqT_st[0:64, :, :])
            nc.sync.dma_start(
                out=_bc(kqT, [kqT.ap[0], [2 * 128, C], [1, 128]], off=128),
                in_=kqT_st[64:128, :, :])

            # ---------------- q prep: int8 quant + csfq fold + transpose ---
            qq, sf_q = quant_int8(q_sb, "q")
            csfq = scales.tile([128, T], f32, tag="csfq")
            nc.vector.tensor_scalar_mul(csfq, sf_q, SM_SCALE)
            qcs = work.tile([128, T, D], f32, tag="qcs")
            nc.vector.tensor_mul(qcs, qq,
                                 _bc(csfq, [csfq.ap[0], [1, T], [0, D]]))
            qcs_h = work.tile([128, T, D], f16, tag="qcs_h")
            nc.vector.tensor_copy(qcs_h, qcs)
            qcsT_st = work.tile([128, C, 128], f16, tag="qcsT_st")
            qcsT = opnds.tile([64, T, 128], f16, tag="qcsT")
            for c in range(C):
                tp = ps_t.tile([128, 128], f16, tag="pst")
                nc.tensor.transpose(tp, qcs_h[:, 2 * c:2 * c + 2, :], ident_h)
                nc.vector.tensor_copy(qcsT_st[:, c, :], tp)
            nc.sync.dma_start(out=_bc(qcsT, [qcsT.ap[0], [2 * 128, C], [1, 128]]),
                              in_=qcsT_st[0:64, :, :])
            nc.sync.dma_start(
                out=_bc(qcsT, [qcsT.ap[0], [2 * 128, C], [1, 128]], off=128),
                in_=qcsT_st[64:128, :, :])

            if stage == "prep":
                # smoke-test prep only: dump quantized operands to out
                nc.sync.dma_start(
                    out=o_d[h].rearrange("(t p) d -> p t d", p=128), in_=kq)
                continue

            # ---------------- main attention loop --------------------------
            TH = T // 2  # t-tiles per query half
            for half in range(2):
                o_ps = ps_o.tile([65, NQH], f32, tag="pso")
                for mt in range(T):
                    s_ps = ps_s.tile([128, NQH], f32, tag="pss")
                    lhsT = kqT[:, mt, :]
                    for j in range(NQH // 512):
                        rhs = qcsT[:, half * TH + 4 * j: half * TH + 4 * (j + 1), :]
                        nc.tensor.matmul(
                            s_ps[:, j * 512:(j + 1) * 512],
                            lhsT, rhs,
                            start=True, stop=True)
                    p_sb = pbuf.tile([128, NQH], f16, tag="p_sb")
                    if stage == "noexp":
                        nc.vector.tensor_copy(p_sb, s_ps)
                    else:
                        nc.scalar.activation(p_sb, s_ps, ACTF.Exp,
                                             scale=sf_k[:, mt:mt + 1])
                    for j in range(NQH // 512):
                        nc.tensor.matmul(
                            o_ps[:, j * 512:(j + 1) * 512],
                            vq_aug[:, mt, :],
                            p_sb[:, j * 512:(j + 1) * 512],
                            start=(mt == 0), stop=(mt == T - 1))
                # finalize this query half
                outT_sb = obuf.tile([65, NQH], f32, tag="outT")
                nc.vector.tensor_scalar_mul(outT_sb, o_ps, sfv65[:, 0:1])
                if stage in ("nofin", "noexp"):
                    nc.sync.dma_start(
                        out=o_d[h, half * NQH:(half + 1) * NQH, :]
                        .rearrange("n d -> d n"),
                        in_=outT_sb[0:D, :])
                    continue
                for c in range(NQH // 128):
                    tp2 = ps_t.tile([128, 65], f32, tag="pst")
                    nc.tensor.transpose(tp2, outT_sb[:, c * 128:(c + 1) * 128],
                                        ident_f[0:65, 0:65])
                    rec = ostore.tile([128, 1], f32, tag="rec")
                    nc.vector.reciprocal(rec, tp2[:, D:D + 1])
                    outc = ostore.tile([128, D], f32, tag="outc")
                    nc.vector.tensor_scalar_mul(outc, tp2[:, 0:D], rec[:, 0:1])
                    row0 = half * NQH + c * 128
                    nc.sync.dma_start(out=o_d[h, row0:row0 + 128, :], in_=outc)
    return nc


_CACHED = {}


def _get_nc():
    if "nc" not in _CACHED:
        from concourse import bacc

        nc = bacc.Bacc("TRN2", target_bir_lowering=False, debug=False)
        build_attention(nc)
        nc.compile()
        _CACHED["nc"] = nc
    return _CACHED["nc"]


def kernel(q: np.ndarray, k: np.ndarray, v: np.ndarray) -> np.ndarray:
    from concourse.bass_utils import run_bass_kernel_spmd

    nc = _get_nc()
    qf = np.ascontiguousarray(np.asarray(q, dtype=np.float32).reshape(B * H, N, D))
    kf = np.ascontiguousarray(np.asarray(k, dtype=np.float32).reshape(B * H, N, D))
    vf = np.ascontiguousarray(np.asarray(v, dtype=np.float32).reshape(B * H, N, D))
    hpc = HEADS_PER_CORE
    in_maps = [
        {"q": qf[c * hpc:(c + 1) * hpc],
         "k": kf[c * hpc:(c + 1) * hpc],
         "v": vf[c * hpc:(c + 1) * hpc]}
        for c in range(NCORES)
    ]
    res = run_bass_kernel_spmd(nc, in_maps, core_ids=list(range(NCORES)))
    out = np.concatenate([np.asarray(r["out"]) for r in res.results], axis=0)
    return out.reshape(B, H, N, D).astype(np.float32)


# revision 25
# speedup vs baseline: 1.0866x; 1.0866x over previous
"""Trainium2 Bass kernel for Ac4kAttentionOp (int8 q/k + fp8e4m3 v quantized attention).

Shapes: q,k,v [B=2, H=16, N=2048, D=64] fp32 -> out [2,16,2048,64] fp32.
Sharding: 32 (B,H) heads split 4-per-core across 8 NeuronCores; no collectives.

Math (mirrors the reference exactly up to fp32 rounding order):
  k <- k - mean_N(k)
  qq = round(q / sf_q), sf_q = max(amax_D(q)/127, eps)      (per token)
  kq = round(k / sf_k), sf_k = max(amax_D(k)/127, eps)      (per token)
  vq = fp8e4m3(v / sf_v), sf_v = max(amax_N(v)/(448/2.25), eps)  (per channel)
  s^T[m,nq] = sum_d kq[m,d] * (qq[nq,d]*sf_q[nq]*sm) ;  p^T = exp(sf_k[m] * s^T)
  outT[d,nq] = sum_m vq[m,d] * p^T[m,nq] ; denom = ones-column of vq_aug
  out[nq,d] = outT[d,nq] * sf_v[d] / denom[nq]

Engine mapping: integer-valued operands ride bf16 (exact); fp32-valued moving
operands ride float32r (1 cyc/row at free>=256). Per-query dequant scale is
pre-folded into the QK moving operand; per-key scale applied via the ACT Exp
per-partition scale; per-channel v scale applied per-partition on outT.
"""
import math
from contextlib import ExitStack

import numpy as np

import concourse.bass as bass
import concourse.tile as tile
from concourse import mybir
from concourse.masks import make_identity

B, H, N, D = 2, 16, 2048, 64
NCORES = 8
HEADS_PER_CORE = (B * H) // NCORES          # 4
SM_SCALE = 1.0 / math.sqrt(D)               # 0.125 (exact power of 2)
MAGIC = 12582912.0                          # 1.5*2^23: fp32 RNE integer round
INT8_MAX = 127.0
F8_AMAX_DIV = float(np.float32(448.0) / np.float32(2.25))  # FP8_MAX / MAX_SCALE
EPS = 1e-8

f32 = mybir.dt.float32
f32r = mybir.dt.float32r
bf16 = mybir.dt.bfloat16
f16 = mybir.dt.float16
f8e4 = mybir.dt.float8e4
ALU = mybir.AluOpType
ACTF = mybir.ActivationFunctionType


def _bc(t: bass.AP, dims, off: int = 0) -> bass.AP:
    """Build a broadcast/restrided view of a tile AP (off in elements)."""
    return bass.AP(tensor=t.tensor, offset=t.offset + off, ap=dims)


def build_attention(nc: bass.Bass, heads: int = HEADS_PER_CORE, n: int = N,
                    stage: str = "full", bench_loops: int = 0):
    T = n // 128          # token tiles per head
    C = T // 2            # 128-wide transpose chunks
    NQH = n // 2          # query-half width (PSUM budget)
    q_d = nc.dram_tensor("q", [heads, n, D], f32, kind="ExternalInput").ap()
    k_d = nc.dram_tensor("k", [heads, n, D], f32, kind="ExternalInput").ap()
    v_d = nc.dram_tensor("v", [heads, n, D], f32, kind="ExternalInput").ap()
    o_d = nc.dram_tensor("out", [heads, n, D], f32, kind="ExternalOutput").ap()

    with tile.TileContext(nc) as tc, ExitStack() as ctx:
        singles = ctx.enter_context(tc.tile_pool(name="singles", bufs=1))
        loads = ctx.enter_context(tc.tile_pool(name="loads", bufs=2))
        work = ctx.enter_context(tc.tile_pool(name="work", bufs=2))
        scales = ctx.enter_context(tc.tile_pool(name="scales", bufs=2))
        small = ctx.enter_context(tc.tile_pool(name="small", bufs=4))
        opnds = ctx.enter_context(tc.tile_pool(name="opnds", bufs=2))
        pbuf = ctx.enter_context(tc.tile_pool(name="pbuf", bufs=3))
        obuf = ctx.enter_context(tc.tile_pool(name="obuf", bufs=2))
        ostore = ctx.enter_context(tc.tile_pool(name="ostore", bufs=4))
        ps_s = ctx.enter_context(tc.tile_pool(name="ps_s", bufs=2, space="PSUM"))
        ps_o = ctx.enter_context(tc.tile_pool(name="ps_o", bufs=1, space="PSUM"))
        ps_t = ctx.enter_context(tc.tile_pool(name="ps_t", bufs=2, space="PSUM"))

        ident_f = singles.tile([128, 128], f32)
        make_identity(nc, ident_f)
        ident_h = singles.tile([128, 128], f16)
        make_identity(nc, ident_h)
        ones_row = singles.tile([1, 128], f32)
        nc.gpsimd.memset(ones_row, 1.0)
        # constant [128,128] of 1/n: k-mean matmul weights (exact 2^-11 scale)
        invn_sq = singles.tile([128, 128], f32)
        nc.gpsimd.memset(invn_sq, 1.0 / n)

        if bench_loops:
            ctx.enter_context(tc.For_i(0, bench_loops, 1))

        # warm the ACT exp table while head 0's prep runs
        warm = singles.tile([1, 1], f32)
        nc.gpsimd.memset(warm, 0.0)
        nc.scalar.activation(warm, warm, ACTF.Exp)

        def quant_int8(x_sb, tagpfx):
            """per-token int8 quantize: returns (q_rounded_f32, sf [128,T])."""
            amax = scales.tile([128, T], f32, tag=tagpfx + "amax")
            nc.vector.tensor_reduce(out=amax, in_=x_sb,
                                    axis=mybir.AxisListType.X, op=ALU.max,
                                    apply_absolute_value=True)
            sf = scales.tile([128, T], f32, tag=tagpfx + "sf")
            nc.vector.tensor_scalar(out=sf, in0=amax,
                                    scalar1=1.0 / INT8_MAX, scalar2=EPS,
                                    op0=ALU.mult, op1=ALU.max)
            rsf = scales.tile([128, T], f32, tag=tagpfx + "rsf")
            nc.vector.reciprocal(rsf, sf)
            xq = work.tile([128, T, D], f32, tag=tagpfx + "xq")
            nc.vector.tensor_mul(xq, x_sb,
                                 _bc(rsf, [rsf.ap[0], [1, T], [0, D]]))
            # RNE integer round: (x + MAGIC) - MAGIC
            nc.vector.tensor_scalar(out=xq, in0=xq,
                                    scalar1=MAGIC, scalar2=MAGIC,
                                    op0=ALU.add, op1=ALU.subtract)
            return xq, sf

        def transpose_split(x_h, dstT, tag):
            """[128,(T,64)] fp16 -> [64,(T,128)] via PE chunks + parity DMAs."""
            stk = work.tile([128, C, 128], f16, tag=tag + "_st")
            for c in range(C):
                tp = ps_t.tile([128, 128], f16, tag="pst")
                nc.tensor.transpose(tp, x_h[:, 2 * c:2 * c + 2, :], ident_h)
                nc.vector.tensor_copy(stk[:, c, :], tp)
            nc.sync.dma_start(
                out=_bc(dstT, [dstT.ap[0], [2 * 128, C], [1, 128]]),
                in_=stk[0:64, :, :])
            nc.sync.dma_start(
                out=_bc(dstT, [dstT.ap[0], [2 * 128, C], [1, 128]], off=128),
                in_=stk[64:128, :, :])

        def prep(h):
            st = {}
            # loads: [n, D] -> [128, T, D]
            q_sb = loads.tile([128, T, D], f32, tag="q_sb")
            nc.sync.dma_start(out=q_sb,
                              in_=q_d[h].rearrange("(t p) d -> p t d", p=128))
            k_sb = loads.tile([128, T, D], f32, tag="k_sb")
            nc.sync.dma_start(out=k_sb,
                              in_=k_d[h].rearrange("(t p) d -> p t d", p=128))
            v_sb = loads.tile([128, T, D], f32, tag="v_sb")
            nc.sync.dma_start(out=v_sb,
                              in_=v_d[h].rearrange("(t p) d -> p t d", p=128))

            # ---- v: per-channel fp8 quant ----
            amax_vp = work.tile([128, D], f32, tag="amax_vp")
            nc.vector.tensor_reduce(
                out=amax_vp,
                in_=_bc(v_sb, [v_sb.ap[0], [1, D], [D, T]]),
                axis=mybir.AxisListType.X, op=ALU.max,
                apply_absolute_value=True)
            vt_ps = ps_t.tile([D, 128], f32, tag="pst")
            nc.tensor.transpose(vt_ps, amax_vp, ident_f)
            amax_vT = scales.tile([D, 1], f32, tag="amax_vT")
            nc.vector.tensor_reduce(out=amax_vT, in_=vt_ps,
                                    axis=mybir.AxisListType.X, op=ALU.max)
            sf_vT = scales.tile([D, 1], f32, tag="sf_vT")
            nc.vector.tensor_scalar(out=sf_vT, in0=amax_vT,
                                    scalar1=1.0 / F8_AMAX_DIV, scalar2=EPS,
                                    op0=ALU.mult, op1=ALU.max)
            rsf_vT = scales.tile([D, 1], f32, tag="rsf_vT")
            nc.vector.reciprocal(rsf_vT, sf_vT)
            sfv65 = scales.tile([65, 1], f32, tag="sfv65")
            nc.gpsimd.memset(sfv65, 1.0)
            nc.vector.tensor_copy(sfv65[0:D, :], sf_vT)
            rsf_row = small.tile([1, D], f32, tag="rsf_row")
            nc.sync.dma_start(out=rsf_row, in_=rsf_vT)
            rsf_bps = ps_t.tile([128, D], f32, tag="pst")
            nc.tensor.matmul(rsf_bps, ones_row, rsf_row, start=True, stop=True)
            rsf_b = small.tile([128, D], f32, tag="rsf_b")
            nc.vector.tensor_copy(rsf_b, rsf_bps)
            vq_pre = work.tile([128, T, D], f32, tag="vq_pre")
            nc.vector.tensor_mul(vq_pre, v_sb,
                                 _bc(rsf_b, [rsf_b.ap[0], [0, T], [1, D]]))
            vq_f8 = work.tile([128, T, D], f8e4, tag="vq_f8")
            nc.vector.tensor_copy(vq_f8, vq_pre)
            vq_aug = opnds.tile([128, T, D + 1], f16, tag="vq_aug")
            nc.vector.tensor_copy(vq_aug[:, :, 0:D], vq_f8)
            nc.gpsimd.memset(vq_aug[:, :, D:D + 1], 1.0)

            # ---- k: mean-sub + int8 quant + transpose ----
            meanb_ps = ps_t.tile([128, D], f32, tag="pst")
            for t in range(T):
                nc.tensor.matmul(meanb_ps, invn_sq, k_sb[:, t, :],
                                 start=(t == 0), stop=(t == T - 1))
            meanb = small.tile([128, D], f32, tag="meanb")
            nc.vector.tensor_copy(meanb, meanb_ps)
            ks = work.tile([128, T, D], f32, tag="ks")
            nc.vector.tensor_sub(ks, k_sb,
                                 _bc(meanb, [meanb.ap[0], [0, T], [1, D]]))
            kq, sf_k = quant_int8(ks, "k")
            kq_h = work.tile([128, T, D], f16, tag="kq_h")
            nc.vector.tensor_copy(kq_h, kq)
            kqT = opnds.tile([64, T, 128], f16, tag="kqT")
            transpose_split(kq_h, kqT, "kqT")

            # ---- q: int8 quant + csfq fold + transpose ----
            qq, sf_q = quant_int8(q_sb, "q")
            csfq = scales.tile([128, T], f32, tag="csfq")
            nc.vector.tensor_scalar_mul(csfq, sf_q, SM_SCALE)
            qcs = work.tile([128, T, D], f32, tag="qcs")
            nc.vector.tensor_mul(qcs, qq,
                                 _bc(csfq, [csfq.ap[0], [1, T], [0, D]]))
            qcs_h = work.tile([128, T, D], f16, tag="qcs_h")
            nc.vector.tensor_copy(qcs_h, qcs)
            qcsT = opnds.tile([64, T, 128], f16, tag="qcsT")
            transpose_split(qcs_h, qcsT, "qcsT")

            st.update(kqT=kqT, qcsT=qcsT, vq_aug=vq_aug, sf_k=sf_k,
                      sfv65=sfv65, kq=kq)
            return st

        def main(h, st):
            kqT, qcsT, vq_aug = st["kqT"], st["qcsT"], st["vq_aug"]
            sf_k, sfv65 = st["sf_k"], st["sfv65"]
            if stage == "prep":
                nc.sync.dma_start(
                    out=o_d[h].rearrange("(t p) d -> p t d", p=128),
                    in_=st["kq"])
                return
            TH = T // 2
            for half in range(2):
                o_ps = ps_o.tile([65, NQH], f32, tag="pso")
                for mt in range(T):
                    s_ps = ps_s.tile([128, NQH], f32, tag="pss")
                    lhsT = kqT[:, mt, :]
                    for j in range(NQH // 512):
                        rhs = qcsT[:, half * TH + 4 * j:
                                   half * TH + 4 * (j + 1), :]
                        nc.tensor.matmul(s_ps[:, j * 512:(j + 1) * 512],
                                         lhsT, rhs, start=True, stop=True)
                    p_sb = pbuf.tile([128, NQH], f16, tag="p_sb")
                    if stage == "noexp":
                        nc.vector.tensor_copy(p_sb, s_ps)
                    else:
                        nc.scalar.activation(p_sb, s_ps, ACTF.Exp,
                                             scale=sf_k[:, mt:mt + 1])
                    for j in range(NQH // 512):
                        nc.tensor.matmul(
                            o_ps[:, j * 512:(j + 1) * 512],
                            vq_aug[:, mt, :],
                            p_sb[:, j * 512:(j + 1) * 512],
                            start=(mt == 0), stop=(mt == T - 1))
                outT_sb = obuf.tile([65, NQH], f32, tag="outT")
                nc.vector.tensor_scalar_mul(outT_sb, o_ps, sfv65[:, 0:1])
                if stage in ("nofin", "noexp"):
                    nc.sync.dma_start(
                        out=o_d[h, half * NQH:(half + 1) * NQH, :]
                        .rearrange("n d -> d n"),
                        in_=outT_sb[0:D, :])
                    continue
                for c in range(NQH // 128):
                    tp2 = ps_t.tile([128, 65], f32, tag="pst")
                    nc.tensor.transpose(tp2, outT_sb[:, c * 128:(c + 1) * 128],
                                        ident_f[0:65, 0:65])
                    rec = ostore.tile([128, 1], f32, tag="rec")
                    nc.vector.reciprocal(rec, tp2[:, D:D + 1])
                    outc = ostore.tile([128, D], f32, tag="outc")
                    nc.vector.tensor_scalar_mul(outc, tp2[:, 0:D], rec[:, 0:1])
                    row0 = half * NQH + c * 128
                    nc.sync.dma_start(out=o_d[h, row0:row0 + 128, :], in_=outc)

        # software-pipeline heads: prep(h+1) is emitted before main(h) so the
        # in-order engines overlap next-head prep with this head's attention.
        st = prep(0)
        for h in range(heads):
            st_next = prep(h + 1) if h + 1 < heads else None
            main(h, st)
            st = st_next
    return nc


_CACHED = {}


def _get_nc():
    if "nc" not in _CACHED:
        from concourse import bacc

        nc = bacc.Bacc("TRN2", target_bir_lowering=False, debug=False)
        build_attention(nc)
        nc.compile()
        _CACHED["nc"] = nc
    return _CACHED["nc"]


def kernel(q: np.ndarray, k: np.ndarray, v: np.ndarray) -> np.ndarray:
    from concourse.bass_utils import run_bass_kernel_spmd

    nc = _get_nc()
    qf = np.ascontiguousarray(np.asarray(q, dtype=np.float32).reshape(B * H, N, D))
    kf = np.ascontiguousarray(np.asarray(k, dtype=np.float32).reshape(B * H, N, D))
    vf = np.ascontiguousarray(np.asarray(v, dtype=np.float32).reshape(B * H, N, D))
    hpc = HEADS_PER_CORE
    in_maps = [
        {"q": qf[c * hpc:(c + 1) * hpc],
         "k": kf[c * hpc:(c + 1) * hpc],
         "v": vf[c * hpc:(c + 1) * hpc]}
        for c in range(NCORES)
    ]
    res = run_bass_kernel_spmd(nc, in_maps, core_ids=list(range(NCORES)))
    out = np.concatenate([np.asarray(r["out"]) for r in res.results], axis=0)
    return out.reshape(B, H, N, D).astype(np.float32)


# revision 26
# speedup vs baseline: 1.4308x; 1.3168x over previous
"""Trainium2 Bass kernel for Ac4kAttentionOp (int8 q/k + fp8e4m3 v quantized attention).

Shapes: q,k,v [B=2, H=16, N=2048, D=64] fp32 -> out [2,16,2048,64] fp32.
Sharding: 32 (B,H) heads split 4-per-core across 8 NeuronCores; no collectives.

Math (mirrors the reference exactly up to fp32 rounding order):
  k <- k - mean_N(k)
  qq = round(q / sf_q), sf_q = max(amax_D(q)/127, eps)      (per token)
  kq = round(k / sf_k), sf_k = max(amax_D(k)/127, eps)      (per token)
  vq = fp8e4m3(v / sf_v), sf_v = max(amax_N(v)/(448/2.25), eps)  (per channel)
  s^T[m,nq] = sum_d kq[m,d] * (qq[nq,d]*sf_q[nq]*sm) ;  p^T = exp(sf_k[m] * s^T)
  outT[d,nq] = sum_m vq[m,d] * p^T[m,nq] ; denom = ones-column of vq_aug
  out[nq,d] = outT[d,nq] * sf_v[d] / denom[nq]

Engine mapping: integer-valued operands ride bf16 (exact); fp32-valued moving
operands ride float32r (1 cyc/row at free>=256). Per-query dequant scale is
pre-folded into the QK moving operand; per-key scale applied via the ACT Exp
per-partition scale; per-channel v scale applied per-partition on outT.
"""
import math
from contextlib import ExitStack

import numpy as np

import concourse.bass as bass
import concourse.tile as tile
from concourse import mybir
from concourse.masks import make_identity

B, H, N, D = 2, 16, 2048, 64
NCORES = 8
HEADS_PER_CORE = (B * H) // NCORES          # 4
SM_SCALE = 1.0 / math.sqrt(D)               # 0.125 (exact power of 2)
MAGIC = 12582912.0                          # 1.5*2^23: fp32 RNE integer round
INT8_MAX = 127.0
F8_AMAX_DIV = float(np.float32(448.0) / np.float32(2.25))  # FP8_MAX / MAX_SCALE
EPS = 1e-8

f32 = mybir.dt.float32
f32r = mybir.dt.float32r
bf16 = mybir.dt.bfloat16
f16 = mybir.dt.float16
f8e4 = mybir.dt.float8e4
ALU = mybir.AluOpType
ACTF = mybir.ActivationFunctionType


def _bc(t: bass.AP, dims, off: int = 0) -> bass.AP:
    """Build a broadcast/restrided view of a tile AP (off in elements)."""
    return bass.AP(tensor=t.tensor, offset=t.offset + off, ap=dims)


def build_attention(nc: bass.Bass, heads: int = HEADS_PER_CORE, n: int = N,
                    stage: str = "full", bench_loops: int = 0):
    T = n // 128          # token tiles per head
    C = T // 2            # 128-wide transpose chunks
    NQH = n // 2          # query-half width (PSUM budget)
    q_d = nc.dram_tensor("q", [heads, n, D], f32, kind="ExternalInput").ap()
    k_d = nc.dram_tensor("k", [heads, n, D], f32, kind="ExternalInput").ap()
    v_d = nc.dram_tensor("v", [heads, n, D], f32, kind="ExternalInput").ap()
    o_d = nc.dram_tensor("out", [heads, n, D], f32, kind="ExternalOutput").ap()

    with tile.TileContext(nc) as tc, ExitStack() as ctx:
        singles = ctx.enter_context(tc.tile_pool(name="singles", bufs=1))
        loads = ctx.enter_context(tc.tile_pool(name="loads", bufs=2))
        work = ctx.enter_context(tc.tile_pool(name="work", bufs=2))
        scales = ctx.enter_context(tc.tile_pool(name="scales", bufs=2))
        small = ctx.enter_context(tc.tile_pool(name="small", bufs=4))
        opnds = ctx.enter_context(tc.tile_pool(name="opnds", bufs=2))
        pbuf = ctx.enter_context(tc.tile_pool(name="pbuf", bufs=4))
        obuf = ctx.enter_context(tc.tile_pool(name="obuf", bufs=2))
        ostore = ctx.enter_context(tc.tile_pool(name="ostore", bufs=4))
        ps_s = ctx.enter_context(tc.tile_pool(name="ps_s", bufs=2, space="PSUM"))
        ps_o = ctx.enter_context(tc.tile_pool(name="ps_o", bufs=1, space="PSUM"))
        ps_t = ctx.enter_context(tc.tile_pool(name="ps_t", bufs=2, space="PSUM"))

        ident_f = singles.tile([128, 128], f32)
        make_identity(nc, ident_f)
        ident_h = singles.tile([128, 128], f16)
        make_identity(nc, ident_h)
        ones_row = singles.tile([1, 128], f32)
        nc.gpsimd.memset(ones_row, 1.0)
        # constant [128,128] of 1/n: k-mean matmul weights (exact 2^-11 scale)
        invn_sq = singles.tile([128, 128], f32)
        nc.gpsimd.memset(invn_sq, 1.0 / n)

        if bench_loops:
            ctx.enter_context(tc.For_i(0, bench_loops, 1))

        # warm the ACT exp table while head 0's prep runs
        warm = singles.tile([1, 1], f32)
        nc.gpsimd.memset(warm, 0.0)
        nc.scalar.activation(warm, warm, ACTF.Exp)

        def quant_int8(x_sb, tagpfx):
            """per-token int8 quantize: returns (q_rounded_f32, sf [128,T])."""
            amax = scales.tile([128, T], f32, tag=tagpfx + "amax")
            nc.vector.tensor_reduce(out=amax, in_=x_sb,
                                    axis=mybir.AxisListType.X, op=ALU.max,
                                    apply_absolute_value=True)
            sf = scales.tile([128, T], f32, tag=tagpfx + "sf")
            nc.vector.tensor_scalar(out=sf, in0=amax,
                                    scalar1=1.0 / INT8_MAX, scalar2=EPS,
                                    op0=ALU.mult, op1=ALU.max)
            rsf = scales.tile([128, T], f32, tag=tagpfx + "rsf")
            nc.vector.reciprocal(rsf, sf)
            xq = work.tile([128, T, D], f32, tag=tagpfx + "xq")
            nc.vector.tensor_mul(xq, x_sb,
                                 _bc(rsf, [rsf.ap[0], [1, T], [0, D]]))
            # RNE integer round: (x + MAGIC) - MAGIC
            nc.vector.tensor_scalar(out=xq, in0=xq,
                                    scalar1=MAGIC, scalar2=MAGIC,
                                    op0=ALU.add, op1=ALU.subtract)
            return xq, sf

        def transpose_split(x_h, dstT, tag):
            """[128,(T,64)] fp16 -> [64,(T,128)] via PE chunks + parity DMAs."""
            stk = work.tile([128, C, 128], f16, tag=tag + "_st")
            for c in range(C):
                tp = ps_t.tile([128, 128], f16, tag="pst")
                nc.tensor.transpose(tp, x_h[:, 2 * c:2 * c + 2, :], ident_h)
                nc.vector.tensor_copy(stk[:, c, :], tp)
            nc.sync.dma_start(
                out=_bc(dstT, [dstT.ap[0], [2 * 128, C], [1, 128]]),
                in_=stk[0:64, :, :])
            nc.sync.dma_start(
                out=_bc(dstT, [dstT.ap[0], [2 * 128, C], [1, 128]], off=128),
                in_=stk[64:128, :, :])

        def prep(h):
            st = {}
            # loads: [n, D] -> [128, T, D]
            q_sb = loads.tile([128, T, D], f32, tag="q_sb")
            nc.sync.dma_start(out=q_sb,
                              in_=q_d[h].rearrange("(t p) d -> p t d", p=128))
            k_sb = loads.tile([128, T, D], f32, tag="k_sb")
            nc.sync.dma_start(out=k_sb,
                              in_=k_d[h].rearrange("(t p) d -> p t d", p=128))
            v_sb = loads.tile([128, T, D], f32, tag="v_sb")
            nc.sync.dma_start(out=v_sb,
                              in_=v_d[h].rearrange("(t p) d -> p t d", p=128))

            # ---- k: mean-sub + int8 quant + transpose ----
            meanb_ps = ps_t.tile([128, D], f32, tag="pst")
            for t in range(T):
                nc.tensor.matmul(meanb_ps, invn_sq, k_sb[:, t, :],
                                 start=(t == 0), stop=(t == T - 1))
            meanb = small.tile([128, D], f32, tag="meanb")
            nc.vector.tensor_copy(meanb, meanb_ps)
            ks = work.tile([128, T, D], f32, tag="ks")
            nc.vector.tensor_sub(ks, k_sb,
                                 _bc(meanb, [meanb.ap[0], [0, T], [1, D]]))
            kq, sf_k = quant_int8(ks, "k")
            kq_h = work.tile([128, T, D], f16, tag="kq_h")
            nc.vector.tensor_copy(kq_h, kq)
            kqT = opnds.tile([64, T, 128], f16, tag="kqT")
            transpose_split(kq_h, kqT, "kqT")

            # ---- q: int8 quant + csfq fold + transpose ----
            qq, sf_q = quant_int8(q_sb, "q")
            csfq = scales.tile([128, T], f32, tag="csfq")
            nc.vector.tensor_scalar_mul(csfq, sf_q, SM_SCALE)
            qcs = work.tile([128, T, D], f32, tag="qcs")
            nc.vector.tensor_mul(qcs, qq,
                                 _bc(csfq, [csfq.ap[0], [1, T], [0, D]]))
            qcs_h = work.tile([128, T, D], f16, tag="qcs_h")
            nc.vector.tensor_copy(qcs_h, qcs)
            qcsT = opnds.tile([64, T, 128], f16, tag="qcsT")
            transpose_split(qcs_h, qcsT, "qcsT")

            # ---- v: per-channel fp8 quant ----
            amax_vp = work.tile([128, D], f32, tag="amax_vp")
            nc.vector.tensor_reduce(
                out=amax_vp,
                in_=_bc(v_sb, [v_sb.ap[0], [1, D], [D, T]]),
                axis=mybir.AxisListType.X, op=ALU.max,
                apply_absolute_value=True)
            vt_ps = ps_t.tile([D, 128], f32, tag="pst")
            nc.tensor.transpose(vt_ps, amax_vp, ident_f)
            amax_vT = scales.tile([D, 1], f32, tag="amax_vT")
            nc.vector.tensor_reduce(out=amax_vT, in_=vt_ps,
                                    axis=mybir.AxisListType.X, op=ALU.max)
            sf_vT = scales.tile([D, 1], f32, tag="sf_vT")
            nc.vector.tensor_scalar(out=sf_vT, in0=amax_vT,
                                    scalar1=1.0 / F8_AMAX_DIV, scalar2=EPS,
                                    op0=ALU.mult, op1=ALU.max)
            rsf_vT = scales.tile([D, 1], f32, tag="rsf_vT")
            nc.vector.reciprocal(rsf_vT, sf_vT)
            sfv65 = scales.tile([65, 1], f32, tag="sfv65")
            nc.gpsimd.memset(sfv65, 1.0)
            nc.vector.tensor_copy(sfv65[0:D, :], sf_vT)
            rsf_row = small.tile([1, D], f32, tag="rsf_row")
            nc.sync.dma_start(out=rsf_row, in_=rsf_vT)
            rsf_bps = ps_t.tile([128, D], f32, tag="pst")
            nc.tensor.matmul(rsf_bps, ones_row, rsf_row, start=True, stop=True)
            rsf_b = small.tile([128, D], f32, tag="rsf_b")
            nc.vector.tensor_copy(rsf_b, rsf_bps)
            vq_pre = work.tile([128, T, D], f32, tag="vq_pre")
            nc.vector.tensor_mul(vq_pre, v_sb,
                                 _bc(rsf_b, [rsf_b.ap[0], [0, T], [1, D]]))
            vq_f8 = work.tile([128, T, D], f8e4, tag="vq_f8")
            nc.vector.tensor_copy(vq_f8, vq_pre)
            vq_aug = opnds.tile([128, T, D + 1], f16, tag="vq_aug")
            nc.vector.tensor_copy(vq_aug[:, :, 0:D], vq_f8)
            nc.gpsimd.memset(vq_aug[:, :, D:D + 1], 1.0)

            st.update(kqT=kqT, qcsT=qcsT, vq_aug=vq_aug, sf_k=sf_k,
                      sfv65=sfv65, kq=kq)
            return st

        def main(h, st):
            kqT, qcsT, vq_aug = st["kqT"], st["qcsT"], st["vq_aug"]
            sf_k, sfv65 = st["sf_k"], st["sfv65"]
            if stage == "prep":
                nc.sync.dma_start(
                    out=o_d[h].rearrange("(t p) d -> p t d", p=128),
                    in_=st["kq"])
                return
            TH = T // 2
            for half in range(2):
                o_ps = ps_o.tile([65, NQH], f32, tag="pso")
                for mt in range(T):
                    s_ps = ps_s.tile([128, NQH], f32, tag="pss")
                    lhsT = kqT[:, mt, :]
                    for j in range(NQH // 512):
                        rhs = qcsT[:, half * TH + 4 * j:
                                   half * TH + 4 * (j + 1), :]
                        nc.tensor.matmul(s_ps[:, j * 512:(j + 1) * 512],
                                         lhsT, rhs, start=True, stop=True)
                    p_sb = pbuf.tile([128, NQH], f16, tag="p_sb")
                    if stage == "noexp":
                        nc.vector.tensor_copy(p_sb, s_ps)
                    else:
                        nc.scalar.activation(p_sb, s_ps, ACTF.Exp,
                                             scale=sf_k[:, mt:mt + 1])
                    for j in range(NQH // 512):
                        nc.tensor.matmul(
                            o_ps[:, j * 512:(j + 1) * 512],
                            vq_aug[:, mt, :],
                            p_sb[:, j * 512:(j + 1) * 512],
                            start=(mt == 0), stop=(mt == T - 1))
                outT_sb = obuf.tile([65, NQH], f32, tag="outT")
                nc.vector.tensor_scalar_mul(outT_sb, o_ps, sfv65[:, 0:1])
                if stage in ("nofin", "noexp"):
                    nc.sync.dma_start(
                        out=o_d[h, half * NQH:(half + 1) * NQH, :]
                        .rearrange("n d -> d n"),
                        in_=outT_sb[0:D, :])
                    continue
                for c in range(NQH // 128):
                    tp2 = ps_t.tile([128, 65], f32, tag="pst")
                    nc.tensor.transpose(tp2, outT_sb[:, c * 128:(c + 1) * 128],
                                        ident_f[0:65, 0:65])
                    rec = ostore.tile([128, 1], f32, tag="rec")
                    nc.vector.reciprocal(rec, tp2[:, D:D + 1])
                    outc = ostore.tile([128, D], f32, tag="outc")
                    nc.vector.tensor_scalar_mul(outc, tp2[:, 0:D], rec[:, 0:1])
                    row0 = half * NQH + c * 128
                    nc.sync.dma_start(out=o_d[h, row0:row0 + 128, :], in_=outc)

        # software-pipeline heads: prep(h+1) is emitted before main(h) so the
        # in-order engines overlap next-head prep with this head's attention.
        st = prep(0)
        for h in range(heads):
            st_next = prep(h + 1) if h + 1 < heads else None
            main(h, st)
            st = st_next
    return nc


_CACHED = {}


def _get_nc():
    if "nc" not in _CACHED:
        from concourse import bacc

        nc = bacc.Bacc("TRN2", target_bir_lowering=False, debug=False)
        build_attention(nc)
        nc.compile()
        _CACHED["nc"] = nc
    return _CACHED["nc"]


def kernel(q: np.ndarray, k: np.ndarray, v: np.ndarray) -> np.ndarray:
    from concourse.bass_utils import run_bass_kernel_spmd

    nc = _get_nc()
    qf = np.ascontiguousarray(np.asarray(q, dtype=np.float32).reshape(B * H, N, D))
    kf = np.ascontiguousarray(np.asarray(k, dtype=np.float32).reshape(B * H, N, D))
    vf = np.ascontiguousarray(np.asarray(v, dtype=np.float32).reshape(B * H, N, D))
    hpc = HEADS_PER_CORE
    in_maps = [
        {"q": qf[c * hpc:(c + 1) * hpc],
         "k": kf[c * hpc:(c + 1) * hpc],
         "v": vf[c * hpc:(c + 1) * hpc]}
        for c in range(NCORES)
    ]
    res = run_bass_kernel_spmd(nc, in_maps, core_ids=list(range(NCORES)))
    out = np.concatenate([np.asarray(r["out"]) for r in res.results], axis=0)
    return out.reshape(B, H, N, D).astype(np.float32)
